# revision 46
# baseline (speedup 1.0000x reference)
"""Trainium2 Bass kernel for nn_AttentionModule (S=2048, D=4096, H=32, KV=8, HD=128).

Sharding: tensor-parallel over heads across 8 NeuronCores. Core c owns q-heads
4c..4c+3 and kv-head c (GQA groups stay intact). Each core computes RMSNorm
(norm_w folded into weights on host, rstd computed on device), its QKV
projection shard, RoPE, causal attention for its 4 heads, and a partial output
projection against its 512 columns of wo. The host sums the 8 partial outputs
(the "all-reduce" of the tensor-parallel layout).

Dtype strategy (PE matmuls are 1 cycle/row for all 16-bit dtypes; DVE gets 2x
on 2-byte dtypes; DMA traffic halves):
 - fp16 for range-tame/precision-sensitive tensors: hidden, wq/wk/wv/wo,
   cos/sin, qT/kT (score inputs), attnT (normalized), output staging/partials.
 - bf16 for the softmax path (e, eacc, v_nat, att evac, rc broadcast): scores
   reach 18.4 so e spans e^-24..e^18 — fp16 would overflow/denormalize, bf16's
   f32-range handles it with 0.4% element error (averaged out in the matmul).
 - fp32 PSUM accumulation everywhere; the h^2 running sum (rstd) stays fp32 on
   DVE because its error enters coherently per token.

Schedule notes:
 - Phase 1 runs per s-block: QKV accumulation matmuls, then rstd + RoPE.
   PSUM: acc_ps (6 banks, left stack) closes after the last block's
   evacuations and hands its banks to sc/att/sum; misc_ps (2 banks, RIGHT
   stack) survives through the last rope, then hands banks 6-7 to o_ps.
 - emit_attention is software-pipelined per head: chunks A(h), then
   norm C(h-2), then sum B(h-1), then the att-bank evacuation (on ACT, so
   DVE bursts can't hold the att PSUM bank hostage). Within A(h) the
   score-pair loop is itself pipelined: attV of pair p is emitted after the
   scores of pair p+1 so PE isn't queued behind the exp.
 - Fine-grained causal: diagonal t-chunks only compute/mask/accumulate the
   live s-columns (>= 128r), trimming both score and attV matmul rows.
 - Emission order: attn0, attn1, op(sc0-7), attn2, attn3, op(sc8-11),
   op(sc12-15) — each outproj segment only needs already-normalized attnT.
   The final output tile group narrows [4,2,1,1] to shorten the drain tail.
 - Softmax runs in scores-transposed [t, s] layout: denominators accumulated
   on DVE (bf16 adds) + one ones-column matmul per head-block; reciprocal on
   DVE, broadcast back over partitions via a K=1 ones-row matmul.
 - Causal masking: diagonal chunks use affine_select on GPSIMD for s-blocks
   0-1 and a bf16 mask multiply on DVE for later ones.
"""
import sys

sys.path.insert(0, "/opt/trn_rl_repo")

import math
from contextlib import ExitStack

import numpy as np

import bass_rust as _bass_rust
import concourse.bacc as bacc
import concourse.mybir as mybir
import concourse.tile as tile
from concourse.bass_utils import run_bass_kernel_spmd
from concourse.hw_specs import get_activation_tables

F32R = mybir.dt.float32r
F32 = mybir.dt.float32
F16 = mybir.dt.float16
BF16 = mybir.dt.bfloat16
ALU = mybir.AluOpType
ACTF = mybir.ActivationFunctionType

S, D, H, KV, HD = 2048, 4096, 32, 8, 128
NCORES = 8
QH = H // NCORES          # 4 q heads per core
QI = QH * HD              # 512 local q dims
DC = D // 128             # 32 contraction chunks
SB = 512                  # s-block width
NSB = S // SB             # 4 s-blocks
NTC = S // 128            # 16 t-chunks
EPS = 1e-6
THETA = 50000.0
SM_SCALE = 1.0 / math.sqrt(HD)

LAST_EXEC_NS = None
LAST_RESULT = None
_CACHE = {}

# pipeline-depth knobs (tuned via timeline sim)
KNOBS = dict(hb_bufs=8, sq_act=True, t12_bufs=2, expp_bufs=6, qtmp_bufs=7,
             sc_bufs=2, wkv_bufs=2, sqp_bufs=3, hb_dc=2, interleave=True,
             mask_dve=True, csb=1, kv_dc=4, wq_dc=2, wo_cache=True,
             wop_bufs=8, obig_w=4, outb_bufs=4, early_evac=True,
             mask_pool_sb=1, ham_warmup=4, sums_dve=True, t2_pool=False,
             tail_split=True, defer_rope=False, attevac_act=True,
             attn1_first=False, op1_split=True, evac_mix=False, rb_act=True, evac_first=False,
             eacc_pool_sb=-1, tail_psum_dma=False, rope_interleave=False,
             pair_pipe=True, tail_act_dma=False, rstd_lnexp=False,
             evac_mix_last=True, wq_first=True, op_sched='mid')


class _Bacc(bacc.Bacc):
    """Bacc with activation tables reordered so the one set containing
    Exp+Ln+Copy+Square is preferred — avoids per-call ACT table reloads."""

    def insert_act_table_loads(self):
        has_activation = any(
            isinstance(i, mybir.InstActivation)
            for b in self.main_func.blocks
            for i in b.instructions
        )
        if not has_activation:
            return
        tables = list(get_activation_tables(self.m.arch).items())
        tables.sort(key=lambda kv: 0 if kv[0] == "natural_log_exp_and_others" else 1)
        _bass_rust.insert_act_table_loads(self, tables)


def _build(skip_compile=False):
    use_bacc = _Bacc if KNOBS.get("rstd_lnexp", False) else bacc.Bacc
    nc = use_bacc("TRN2", target_bir_lowering=False, debug=False)

    hT_d = nc.dram_tensor("hT", [D, S], F16, kind="ExternalInput")
    wqT_d = nc.dram_tensor("wqT", [D, QI], F16, kind="ExternalInput")
    wkT_d = nc.dram_tensor("wkT", [D, HD], F16, kind="ExternalInput")
    wvT_d = nc.dram_tensor("wvT", [D, HD], F16, kind="ExternalInput")
    woT_d = nc.dram_tensor("woT", [QI, D], F16, kind="ExternalInput")
    cos_d = nc.dram_tensor("cosT", [128, S], F16, kind="ExternalInput")
    sin_d = nc.dram_tensor("sinT", [128, S], F16, kind="ExternalInput")
    prot_d = nc.dram_tensor("protT", [128, 128], F16, kind="ExternalInput")
    ident_d = nc.dram_tensor("ident", [128, 128], BF16, kind="ExternalInput")
    onec_d = nc.dram_tensor("ones_col", [128, 1], F32R, kind="ExternalInput")
    onecb_d = nc.dram_tensor("ones_col_bf", [128, 1], BF16, kind="ExternalInput")
    oner_d = nc.dram_tensor("ones_row", [1, 128], F32R, kind="ExternalInput")
    mask_d = nc.dram_tensor("maskT", [128, 4 * SB], BF16, kind="ExternalInput")
    out_d = nc.dram_tensor("outp", [S, D], F16, kind="ExternalOutput")

    hT3 = hT_d.rearrange("(o p) s -> p o s", p=128)      # [128, 32, 2048]
    wqT3 = wqT_d.rearrange("(o p) i -> p o i", p=128)    # [128, 32, 512]
    wkT3 = wkT_d.rearrange("(o p) e -> p o e", p=128)    # [128, 32, 128]
    wvT3 = wvT_d.rearrange("(o p) e -> p o e", p=128)
    woT3 = woT_d.rearrange("(g p) j -> p g j", p=128)    # [128, 4, 4096]
    out4 = out_d.rearrange("(g p) j -> p g j", p=128)    # [128, 16, 4096]

    HB_DC = KNOBS.get("hb_dc", 2)  # hT chunks per DMA

    with tile.TileContext(nc) as tc:
        with ExitStack() as root:
            consts = root.enter_context(tc.tile_pool(name="consts", bufs=1))
            persist = root.enter_context(tc.tile_pool(name="persist", bufs=1))

            onec_t = consts.tile([128, 1], F32R, tag="onec")
            onecb_t = consts.tile([128, 1], BF16, tag="onecb")
            oner_t = consts.tile([1, 128], F32R, tag="oner")
            eps_t = consts.tile([1, 1], F32, tag="eps")
            nc.vector.memset(eps_t, EPS)

            qT_all = persist.tile([128, QH, S], F16, tag="qT")
            kT_all = persist.tile([128, S], F16, tag="kT")
            v_nat = persist.tile([128, NTC, HD], BF16, tag="vn")

            # rope constants + scratch outlive phase 1 (sb3's rope is
            # deferred into the attention region)
            c1 = root.enter_context(tc.tile_pool(name="c1", bufs=1))
            cos_t = c1.tile([128, S], F16, tag="cos")
            sin_t = c1.tile([128, S], F16, tag="sin")
            prot_t = c1.tile([128, 128], F16, tag="prot")
            ident_t = c1.tile([128, 128], BF16, tag="ident")
            c1_loaded = [False]
            scr2 = root.enter_context(tc.tile_pool(name="scr2", bufs=2))

            # misc_ps on the RIGHT psum stack: outlives acc_ps (left) through
            # the deferred rope, then hands banks 6-7 to o_ps
            misc_stack = ExitStack()
            misc_ps = misc_stack.enter_context(
                tc.tile_pool(name="misc_ps", bufs=2, space="PSUM", side="right")
            )

            # --- rstd/rope helpers (used both in-loop and deferred) ---
            def emit_rstd(sb, sqr, sq_last=None):
                ssl = slice(SB * sb, SB * (sb + 1))
                ms_ps = misc_ps.tile([1, SB], F32, tag="misc", name=f"ms_ps{sb}")
                if sq_last is None:
                    nc.tensor.matmul(ms_ps, onec_t, sqr, start=True, stop=True)
                else:
                    # split: bulk term ready one DVE-add earlier than the tail
                    nc.tensor.matmul(ms_ps, onec_t, sqr, start=True, stop=False)
                    nc.tensor.matmul(ms_ps, onec_t, sq_last, start=False,
                                     stop=True)
                lnt = scr2.tile([1, SB], F32, tag="lnt", bufs=1)
                rstd = scr2.tile([1, SB], F32R, tag="rstd", bufs=1)
                if KNOBS.get("rstd_lnexp", True):
                    # rstd = exp(-0.5 ln(ms/D + eps)): both funcs live in the
                    # natural_log_exp set, and the whole chain stays on ACT so
                    # the rb broadcast never waits on the DVE queue
                    nc.scalar.activation(
                        out=lnt, in_=ms_ps, func=ACTF.Ln, scale=1.0 / D,
                        bias=eps_t
                    )
                    with nc.allow_low_precision(reason="rstd row fp32r"):
                        nc.scalar.activation(
                            out=rstd, in_=lnt, func=ACTF.Exp, scale=-0.5
                        )
                else:
                    nc.scalar.activation(
                        out=lnt, in_=ms_ps, func=ACTF.Sqrt, scale=1.0 / D,
                        bias=eps_t
                    )
                    with nc.allow_low_precision(reason="rstd row fp32r"):
                        nc.vector.reciprocal(out=rstd, in_=lnt.bitcast(F32R))
                rb_ps = misc_ps.tile([128, SB], F32, tag="misc", name=f"rb_ps{sb}")
                nc.tensor.matmul(rb_ps, oner_t, rstd, start=True, stop=True)
                rb_sb = scr2.tile([128, SB], F16, tag="rb_sb",
                                  bufs=KNOBS.get("csb", 2))
                (nc.scalar.copy if KNOBS.get("rb_act", True)
                 else nc.vector.tensor_copy)(out=rb_sb, in_=rb_ps)
                cosrb = scr2.tile([128, SB], F16, tag="cosrb",
                                  bufs=KNOBS.get("csb", 2))
                nc.vector.tensor_tensor(cosrb, cos_t[:, ssl], rb_sb, ALU.mult)
                sinrb = scr2.tile([128, SB], F16, tag="sinrb",
                                  bufs=KNOBS.get("csb", 2))
                nc.vector.tensor_tensor(sinrb, sin_t[:, ssl], rb_sb, ALU.mult)
                return rb_sb, cosrb, sinrb

            def emit_rope_q(sb, i, qtmp, cosrb, sinrb):
                ssl = slice(SB * sb, SB * (sb + 1))
                rot_ps = misc_ps.tile([128, SB], F32, tag="misc",
                                      name=f"rot_q{i}_{sb}")
                nc.tensor.matmul(rot_ps, prot_t, qtmp, start=True, stop=True)
                t1 = scr2.tile([128, SB], F16, tag="t1", bufs=KNOBS["t12_bufs"])
                nc.vector.tensor_tensor(t1, qtmp, cosrb, ALU.mult)
                t2 = scr2.tile([128, SB], F16, tag="t2", bufs=KNOBS["t12_bufs"])
                (nc.gpsimd if KNOBS.get("t2_pool", False) else nc.vector
                 ).tensor_tensor(t2, rot_ps, sinrb, ALU.mult)
                nc.vector.tensor_tensor(qT_all[:, i, ssl], t1, t2, ALU.add)

            def emit_rope_k(sb, ktmp, cosrb, sinrb):
                ssl = slice(SB * sb, SB * (sb + 1))
                rot_ps = misc_ps.tile([128, SB], F32, tag="misc",
                                      name=f"rot_k{sb}")
                nc.tensor.matmul(rot_ps, prot_t, ktmp, start=True, stop=True)
                t1 = scr2.tile([128, SB], F16, tag="t1",
                               bufs=KNOBS["t12_bufs"], name="t1k")
                nc.vector.tensor_tensor(t1, ktmp, cosrb, ALU.mult)
                t2 = scr2.tile([128, SB], F16, tag="t2",
                               bufs=KNOBS["t12_bufs"], name="t2k")
                (nc.gpsimd if KNOBS.get("t2_pool", False) else nc.vector
                 ).tensor_tensor(t2, rot_ps, sinrb, ALU.mult)
                nc.vector.tensor_tensor(kT_all[:, ssl], t1, t2, ALU.add)

            def emit_v(sb, vtmp, rb_sb):
                vsc = scr2.tile([128, SB], BF16, tag="vsc",
                                bufs=KNOBS.get("csb", 2))
                nc.vector.tensor_tensor(vsc, vtmp, rb_sb, ALU.mult)
                for j in range(SB // 128):
                    tcx = (SB // 128) * sb + j
                    vtr_ps = misc_ps.tile([128, 128], BF16, tag="misc",
                                          name=f"vtr{tcx}")
                    nc.tensor.transpose(
                        vtr_ps, vsc[:, 128 * j: 128 * (j + 1)], ident_t
                    )
                    nc.vector.tensor_copy(out=v_nat[:, tcx, :], in_=vtr_ps)

            # ------------- Phase 1: QKV projections + rstd + RoPE -------------
            saved = {}
            with ExitStack() as ph1:
                wqp = ph1.enter_context(tc.tile_pool(name="wqp", bufs=1))
                wq_t = wqp.tile([128, DC, QI], F16, tag="wqr")
                wkvp = ph1.enter_context(tc.tile_pool(name="wkvp", bufs=KNOBS["wkv_bufs"]))
                hb = ph1.enter_context(tc.tile_pool(name="hb", bufs=KNOBS["hb_bufs"]))
                sqp = ph1.enter_context(tc.tile_pool(name="sqp", bufs=KNOBS["sqp_bufs"]))
                scrA = ph1.enter_context(tc.tile_pool(name="scrA", bufs=2))
                if KNOBS.get("acc_split", True):
                    # k/v banks (released last -> att/sum) below q banks
                    # (released first -> sc): scores start after only the
                    # four q evacuations instead of all six
                    acca_st = ExitStack()
                    acc_kv = acca_st.enter_context(
                        tc.tile_pool(name="acc_kv", bufs=1, space="PSUM"))
                    accb_st = ExitStack()
                    acc_q = accb_st.enter_context(
                        tc.tile_pool(name="acc_q", bufs=1, space="PSUM"))
                else:
                    acc_ps = ph1.enter_context(
                        tc.tile_pool(name="acc_ps", bufs=1, space="PSUM")
                    )
                    acc_kv = acc_q = acc_ps
                    acca_st = accb_st = None

                if KNOBS.get("ham_warmup", 0):
                    # PE p-state ramp warm-up: dummy matmuls on a zeroed tile
                    # during the initial DMA wait so real matmuls start at
                    # full clock (the cost model tracks a 3us ramp of
                    # continuous PE activity).
                    wu = scr2.tile([128, SB], F16, tag="qtmp",
                                   bufs=KNOBS["qtmp_bufs"], name="warmup_src")
                    nc.vector.memset(wu, 0.0)
                    wu_ps = misc_ps.tile([128, SB], F32, tag="misc", name="wu_ps")
                    for _w in range(KNOBS["ham_warmup"]):
                        nc.tensor.matmul(wu_ps, wu[:, :128], wu,
                                         start=(_w == 0),
                                         stop=(_w == KNOBS["ham_warmup"] - 1))

                pending_rope = []
                for sb in range(NSB):
                    ssl = slice(SB * sb, SB * (sb + 1))
                    q_ps = [
                        acc_q.tile([128, SB], F32, tag=f"q{i}", name=f"q_ps{i}")
                        for i in range(QH)
                    ]
                    k_ps = acc_kv.tile([128, SB], F32, tag="k")
                    v_ps = acc_kv.tile([128, SB], F32, tag="v")
                    sqacc = scrA.tile([128, SB], F32, tag="sqacc", bufs=2)
                    sqr = scrA.tile([128, SB], F32R, tag="sqr", bufs=1)
                    sq_last = [None]
                    KV_DC = KNOBS.get("kv_dc", 4)  # wk/wv chunk width
                    WQ_DC = KNOBS.get("wq_dc", 2)  # wq load width (sb 0)
                    for hc in range(DC // HB_DC):
                        if sb == 0 and hc == 0 and KNOBS.get("wq_first", True):
                            # first q-matmul needs wq chunk 0: issue it ahead
                            # of the h/wk/wv stream
                            nc.sync.dma_start(out=wq_t[:, 0:WQ_DC, :],
                                              in_=wqT3[:, 0:WQ_DC, :])
                        ht2 = hb.tile([128, HB_DC, SB], F16, tag="h")
                        nc.sync.dma_start(out=ht2, in_=hT3[:, HB_DC*hc:HB_DC*(hc+1), ssl])
                        if (HB_DC * hc) % KV_DC == 0:
                            kc0 = HB_DC * hc
                            wkc = wkvp.tile([128, KV_DC, HD], F16, tag="wk2")
                            nc.sync.dma_start(
                                out=wkc, in_=wkT3[:, kc0:kc0+KV_DC, :])
                            wvc = wkvp.tile([128, KV_DC, HD], F16, tag="wv2")
                            nc.sync.dma_start(
                                out=wvc, in_=wvT3[:, kc0:kc0+KV_DC, :])
                        if (KNOBS.get("rope_interleave", True)
                                and pending_rope and hc % 2 == 0):
                            pending_rope.pop(0)()
                        for j in range(HB_DC):
                            dc = HB_DC * hc + j
                            ht = ht2[:, j, :]
                            if sb == 0 and dc % WQ_DC == 0 and not (
                                    dc == 0 and KNOBS.get("wq_first", True)):
                                nc.sync.dma_start(out=wq_t[:, dc:dc+WQ_DC, :],
                                                  in_=wqT3[:, dc:dc+WQ_DC, :])
                            wqc = wq_t[:, dc, :]
                            if sb == 0 and dc == 8 and not c1_loaded[0]:
                                nc.sync.dma_start(out=cos_t, in_=cos_d[:, :])
                                nc.sync.dma_start(out=sin_t, in_=sin_d[:, :])
                                nc.sync.dma_start(out=prot_t, in_=prot_d[:, :])
                                nc.sync.dma_start(out=ident_t, in_=ident_d[:, :])
                                nc.sync.dma_start(out=onec_t, in_=onec_d[:, :])
                                nc.sync.dma_start(out=onecb_t, in_=onecb_d[:, :])
                                nc.sync.dma_start(out=oner_t, in_=oner_d[:, :])
                                c1_loaded[0] = True
                            sq = sqp.tile([128, SB],
                                          F32R if dc == DC - 1 else F32,
                                          tag="sq")
                            if KNOBS["sq_act"]:
                                nc.scalar.activation(out=sq, in_=ht, func=ACTF.Square)
                            else:
                                nc.vector.tensor_tensor(sq, ht, ht, ALU.mult)
                            if dc == 0:
                                nc.vector.tensor_copy(out=sqacc, in_=sq)
                            elif dc == DC - 1:
                                sq_last[0] = sq
                            elif dc == DC - 2:
                                nc.vector.tensor_tensor(sqr, sqacc, sq, ALU.add)
                            else:
                                nc.vector.tensor_tensor(sqacc, sqacc, sq, ALU.add)
                            for i in range(QH):
                                nc.tensor.matmul(
                                    q_ps[i],
                                    wqc[:, 128 * i: 128 * (i + 1)],
                                    ht,
                                    start=(dc == 0),
                                    stop=(dc == DC - 1),
                                )
                            nc.tensor.matmul(
                                k_ps, wkc[:, dc % KV_DC, :], ht,
                                start=(dc == 0), stop=(dc == DC - 1),
                            )
                            nc.tensor.matmul(
                                v_ps, wvc[:, dc % KV_DC, :], ht,
                                start=(dc == 0), stop=(dc == DC - 1),
                            )
                    # rstd + PSUM evacuation. Evacs are plain copies with no
                    # rstd dependency, so the next s-block's accumulation
                    # starts immediately; rstd is folded into per-block
                    # cos/sin tables instead.
                    if KNOBS.get("evac_first", True):
                        rb_sb = cosrb = sinrb = None  # rstd emitted after evacs
                    else:
                        rb_sb, cosrb, sinrb = emit_rstd(sb, sqr, sq_last[0])
                    mix_here = (KNOBS.get("evac_mix", True)
                                or (sb == NSB - 1
                                    and KNOBS.get("evac_mix_last", True)))
                    if mix_here:
                        # GPSIMD cannot read PSUM: alternate DVE/ACT only
                        evac_engines = [nc.vector.tensor_copy, nc.scalar.copy,
                                        nc.vector.tensor_copy, nc.scalar.copy,
                                        nc.vector.tensor_copy, nc.scalar.copy]
                    else:
                        evac_engines = [nc.vector.tensor_copy] * 6
                    qtmps = []
                    for i in range(QH):
                        qtmp = scr2.tile([128, SB], F16, tag="qtmp",
                                         bufs=KNOBS["qtmp_bufs"])
                        evac_engines[i](out=qtmp, in_=q_ps[i])
                        qtmps.append(qtmp)
                    ktmp = scr2.tile([128, SB], F16, tag="qtmp",
                                     bufs=KNOBS["qtmp_bufs"], name="ktmp")
                    evac_engines[4](out=ktmp, in_=k_ps)
                    vtmp = scr2.tile([128, SB], BF16, tag="qtmp",
                                     bufs=KNOBS["qtmp_bufs"], name="vtmp")
                    evac_engines[5](out=vtmp, in_=v_ps)
                    if KNOBS.get("evac_first", True):
                        rb_sb, cosrb, sinrb = emit_rstd(sb, sqr, sq_last[0])
                    if sb == NSB - 1 and KNOBS.get("defer_rope", True):
                        saved = dict(qtmps=qtmps, ktmp=ktmp, vtmp=vtmp,
                                     rb_sb=rb_sb, cosrb=cosrb, sinrb=sinrb)
                    elif sb < NSB - 1 and KNOBS.get("rope_interleave", True):
                        # spread this block's rope into the next block's
                        # accumulation loop (DVE has slack there)
                        import functools
                        for i in range(QH):
                            pending_rope.append(functools.partial(
                                emit_rope_q, sb, i, qtmps[i], cosrb, sinrb))
                        pending_rope.append(functools.partial(
                            emit_rope_k, sb, ktmp, cosrb, sinrb))
                        pending_rope.append(functools.partial(
                            emit_v, sb, vtmp, rb_sb))
                    else:
                        for i in range(QH):
                            emit_rope_q(sb, i, qtmps[i], cosrb, sinrb)
                        emit_rope_k(sb, ktmp, cosrb, sinrb)
                        emit_v(sb, vtmp, rb_sb)
                for fn in pending_rope:
                    fn()
                pending_rope = []

            # ph1 closed: release q banks (-> sc) then k/v banks (-> att/sum)
            if KNOBS.get("acc_split", True):
                accb_st.close()
                acca_st.close()
            persist2 = root.enter_context(tc.tile_pool(name="persist2", bufs=1))
            attnT = persist2.tile([128, QH, S], F16, tag="attnT")
            mask_t = persist2.tile([128, 4, SB], BF16, tag="mask")
            nc.sync.dma_start(out=mask_t, in_=mask_d.rearrange("p (r s) -> p r s", s=SB))

            outb = root.enter_context(tc.tile_pool(name="outb", bufs=KNOBS.get("outb_bufs", 2)))
            wop = root.enter_context(tc.tile_pool(name="wop", bufs=KNOBS.get("wop_bufs", 2)))

            # ------------- Phase 3+4 interleaved ------------------------------
            ph3 = ExitStack()
            sc_ps_p = ph3.enter_context(
                tc.tile_pool(name="sc_ps", bufs=KNOBS["sc_bufs"], space="PSUM")
            )
            att_ps_p = ph3.enter_context(
                tc.tile_pool(name="att_ps", bufs=KNOBS.get("att_bufs", 1), space="PSUM")
            )
            sum_ps_p = ph3.enter_context(
                tc.tile_pool(name="sum_ps", bufs=1, space="PSUM")
            )
            expp = ph3.enter_context(tc.tile_pool(name="expp", bufs=KNOBS["expp_bufs"]))
            scr3 = ph3.enter_context(tc.tile_pool(name="scr3", bufs=2))

            o_holder = [None]

            def emit_attention(sb, head_cb=None):
                ssl = slice(SB * sb, SB * (sb + 1))
                n_tc = (SB // 128) * (sb + 1)
                SCP = 2 if KNOBS.get("sc_pair", True) else 1
                st = {}  # per-head pipeline state

                def stage_chunks(h):
                    att_ps = att_ps_p.tile([128, SB], F32, tag="att",
                                           name=f"att{h}_{sb}")
                    eacc = scr3.tile([128, SB], BF16, tag="eacc", bufs=2,
                                     name=f"eacc{h}_{sb}")
                    def consume_pair(e_pair, rr):
                        # mask + attV + denominator accumulation for a pair
                        for u in range(len(rr)):
                            tcx, r, csl = rr[u]
                            e_sb = e_pair[:, u, csl]
                            if r >= 0:
                                # diagonal chunk: zero where t > s; dense early
                                # blocks go to idle GPSIMD, late ones to DVE
                                if sb <= KNOBS.get("mask_pool_sb", -1):
                                    nc.gpsimd.affine_select(
                                        e_sb, e_sb,
                                        pattern=[[1, csl.stop - csl.start]],
                                        compare_op=ALU.is_ge,
                                        fill=0.0,
                                        base=-(128 * r) + csl.start,
                                        channel_multiplier=-1,
                                    )
                                else:
                                    nc.vector.tensor_tensor(
                                        e_sb, e_sb, mask_t[:, r, csl], ALU.mult
                                    )
                            nc.tensor.matmul(
                                att_ps[:, csl], v_nat[:, tcx, :], e_sb,
                                start=(tcx == 0), stop=(tcx == n_tc - 1),
                            )
                            if tcx == 0:
                                nc.vector.tensor_copy(out=eacc, in_=e_sb)
                            else:
                                nc.vector.tensor_tensor(
                                    eacc[:, csl], eacc[:, csl], e_sb, ALU.add)

                    pending = []  # (e_pair, rr) whose attV is not yet emitted
                    for tp in range(n_tc // SCP):
                        # paired scores tiles -> one wide exp; diagonal chunks
                        # only compute the live s-columns (>= 128r). attV of
                        # pair p is emitted after scores of pair p+1 so PE
                        # isn't queued behind the exp.
                        sc_ps = sc_ps_p.tile([128, SCP, SB], F32, tag="sc",
                                             name=f"sc{h}_{sb}_{tp}")
                        e_pair = expp.tile([128, SCP, SB], BF16, tag="e",
                                           name=f"e{h}_{sb}_{tp}")
                        rr = []
                        for u in range(SCP):
                            tcx = SCP * tp + u
                            r = tcx - (SB // 128) * sb
                            csl = slice(128 * r, SB) if r > 0 else slice(0, SB)
                            rr.append((tcx, r, csl))
                            nc.tensor.matmul(
                                sc_ps[:, u, csl],
                                kT_all[:, 128 * tcx: 128 * (tcx + 1)],
                                qT_all[:, h, SB * sb + csl.start:
                                       SB * sb + csl.stop],
                                start=True, stop=True,
                            )
                        if rr[0][1] > 0:
                            # all-diagonal pair: exp only the live columns
                            for u in range(SCP):
                                tcx, r, csl = rr[u]
                                nc.scalar.activation(
                                    out=e_pair[:, u, csl], in_=sc_ps[:, u, csl],
                                    func=ACTF.Exp, scale=SM_SCALE
                                )
                        else:
                            nc.scalar.activation(
                                out=e_pair, in_=sc_ps, func=ACTF.Exp,
                                scale=SM_SCALE
                            )
                        if KNOBS.get("pair_pipe", True):
                            pending.append((e_pair, rr))
                            if len(pending) > KNOBS.get("pair_depth", 1):
                                consume_pair(*pending.pop(0))
                        else:
                            consume_pair(e_pair, rr)
                    for pp in pending:
                        consume_pair(*pp)
                    pending = []
                    st[h] = dict(att_ps=att_ps, eaccr=eacc)

                def stage_evac(h):
                    # unnormalized numerator off PSUM (bf16 for range); on ACT
                    # so a busy DVE can't hold the att bank hostage
                    att_sb = scr3.tile([128, SB], BF16, tag="attsb", bufs=2,
                                       name=f"attsb{h}_{sb}")
                    if KNOBS.get("attevac_act", True):
                        nc.scalar.copy(out=att_sb, in_=st[h]["att_ps"])
                    else:
                        nc.vector.tensor_copy(out=att_sb, in_=st[h]["att_ps"])
                    st[h]["att_sb"] = att_sb

                def stage_sum(h):
                    sum_ps = sum_ps_p.tile([1, SB], F32, tag="sumrc",
                                           name=f"sum{h}_{sb}")
                    nc.tensor.matmul(sum_ps, onecb_t, st[h]["eaccr"],
                                     start=True, stop=True)
                    st[h]["sum_ps"] = sum_ps

                def stage_norm(h):
                    rcv = scr3.tile([1, SB], F32R, tag="rcv", bufs=2,
                                    name=f"rcv{h}_{sb}")
                    with nc.allow_low_precision(reason="softmax recip row"):
                        nc.vector.reciprocal(out=rcv,
                                             in_=st[h]["sum_ps"].bitcast(F32R))
                    rc_ps = sum_ps_p.tile([128, SB], F32, tag="sumrc",
                                          name=f"rc{h}_{sb}")
                    nc.tensor.matmul(rc_ps, oner_t, rcv, start=True, stop=True)
                    rc_sb = scr3.tile([128, SB], BF16, tag="rcsb", bufs=2,
                                      name=f"rcsb{h}_{sb}")
                    nc.vector.tensor_copy(out=rc_sb, in_=rc_ps)
                    nc.vector.tensor_tensor(
                        attnT[:, h, ssl], st[h]["att_sb"], rc_sb, ALU.mult
                    )

                for h in range(QH):
                    stage_chunks(h)
                    if h >= 2:
                        stage_norm(h - 2)
                    if h >= 1:
                        stage_sum(h - 1)
                    stage_evac(h)
                    if head_cb is not None:
                        head_cb(h)
                stage_norm(QH - 2)
                stage_sum(QH - 1)
                stage_norm(QH - 1)

            woc_cache = {}

            def emit_outproj(g, sc0=None, nsc=8, jts=None):
                if sc0 is None:
                    sc0 = 8 * g
                OBW = KNOBS.get("obig_w", 4)  # sc-tiles per out staging/DMA
                for jt in (range(D // SB) if jts is None else jts):
                    jsl = slice(SB * jt, SB * (jt + 1))
                    if KNOBS.get("wo_cache", False):
                        if jt not in woc_cache:
                            woc = wop.tile([128, QH, SB], F16, tag="wo",
                                           name=f"wo{jt}")
                            nc.sync.dma_start(out=woc, in_=woT3[:, :, jsl])
                            woc_cache[jt] = woc
                        woc = woc_cache[jt]
                    else:
                        woc = wop.tile([128, QH, SB], F16, tag="wo",
                                       name=f"wo{jt}_{g}")
                        nc.sync.dma_start(out=woc, in_=woT3[:, :, jsl])
                    if sc0 + nsc == 16 and jt == D // SB - 1 and KNOBS.get("tail_split", True):
                        groups = [OBW] * ((nsc - 4) // OBW) + [2, 1, 1]
                    else:
                        assert nsc % OBW == 0 and nsc // OBW > 0, (nsc, OBW)
                        groups = [OBW] * (nsc // OBW)
                    sc_base = sc0
                    for q, gw in enumerate(groups):
                        o_big = outb.tile([128, OBW, SB], F16, tag="obig",
                                          name=f"ob{jt}_{g}_{q}")
                        last_single = (gw == 1 and q == len(groups) - 1
                                       and KNOBS.get("tail_psum_dma", False))
                        for si in range(gw):
                            sc = sc_base + si
                            o_ps = o_holder[0].tile([128, SB], F32, tag="o",
                                                    name=f"o{jt}_{sc}")
                            for h in range(QH):
                                nc.tensor.matmul(
                                    o_ps,
                                    attnT[:, h, 128 * sc: 128 * (sc + 1)],
                                    woc[:, h, :],
                                    start=(h == 0), stop=(h == QH - 1),
                                )
                            if last_single:
                                continue
                            if si % 2 == 0:
                                nc.vector.tensor_copy(out=o_big[:, si, :], in_=o_ps)
                            else:
                                nc.scalar.copy(out=o_big[:, si, :], in_=o_ps)
                        if last_single:
                            nc.sync.dma_start(
                                out=out4[:, sc_base: sc_base + 1, jsl],
                                in_=o_ps,
                            )
                        else:
                            eng = (nc.scalar
                                   if (gw <= 2 and KNOBS.get("tail_act_dma", True))
                                   else nc.sync)
                            eng.dma_start(
                                out=out4[:, sc_base: sc_base + gw, jsl],
                                in_=o_big[:, :gw, :]
                            )
                        sc_base += gw

            if KNOBS.get("defer_rope", True):
                def rope3_head(h):
                    emit_rope_q(NSB - 1, h, saved["qtmps"][h],
                                saved["cosrb"], saved["sinrb"])
                emit_attention(0, head_cb=rope3_head)
                emit_rope_k(NSB - 1, saved["ktmp"], saved["cosrb"],
                            saved["sinrb"])
                emit_v(NSB - 1, saved["vtmp"], saved["rb_sb"])
                misc_stack.close()
                o_holder[0] = root.enter_context(
                    tc.tile_pool(name="o_ps", bufs=2, space="PSUM", side="right")
                )
                emit_attention(1)
            elif KNOBS.get("attn1_first", True):
                # attention(1) has 2x the matmul volume of attention(0):
                # lead with it so sb3's in-loop rope chain hides behind PE work
                emit_attention(1)
                misc_stack.close()
                o_holder[0] = root.enter_context(
                    tc.tile_pool(name="o_ps", bufs=2, space="PSUM", side="right")
                )
                emit_attention(0)
            else:
                emit_attention(0)
                misc_stack.close()
                o_holder[0] = root.enter_context(
                    tc.tile_pool(name="o_ps", bufs=2, space="PSUM", side="right")
                )
                sched = KNOBS.get("op_sched", "late")
            if sched == "early":
                # each outproj segment right after the attention block that
                # completes its attnT rows: fills transition idle with PE work
                emit_outproj(0, sc0=0, nsc=4)
                emit_attention(1)
                emit_outproj(0, sc0=4, nsc=4)
                emit_attention(2)
                emit_outproj(1, sc0=8, nsc=4)
                emit_attention(3)
                emit_outproj(1, sc0=12, nsc=4)
            elif sched == "mid":
                emit_attention(1)
                emit_outproj(0, sc0=0, nsc=4)
                emit_attention(2)
                emit_outproj(0, sc0=4, nsc=4)
                emit_attention(3)
                emit_outproj(1, sc0=8, nsc=4)
                emit_outproj(1, sc0=12, nsc=4)
            elif sched == "fine":
                def op_cb(g, sc0):
                    def cb(h):
                        emit_outproj(g, sc0=sc0, nsc=4, jts=[2 * h, 2 * h + 1])
                    return cb
                emit_attention(1, head_cb=op_cb(0, 0))
                emit_attention(2, head_cb=op_cb(0, 4))
                emit_attention(3, head_cb=op_cb(1, 8))
                emit_outproj(1, sc0=12, nsc=4)
            elif sched == "fine3":
                # op filler callbacks only where attention is ACT-bound
                def op_cb(g, sc0):
                    def cb(h):
                        emit_outproj(g, sc0=sc0, nsc=4, jts=[2 * h, 2 * h + 1])
                    return cb
                emit_attention(1)
                emit_outproj(0, sc0=0, nsc=4)
                emit_attention(2)
                emit_attention(3, head_cb=op_cb(0, 4))
                emit_outproj(1, sc0=8, nsc=4)
                emit_outproj(1, sc0=12, nsc=4)
            elif sched == "fine23":
                def op_cb(g, sc0):
                    def cb(h):
                        emit_outproj(g, sc0=sc0, nsc=4, jts=[2 * h, 2 * h + 1])
                    return cb
                emit_attention(1)
                emit_outproj(0, sc0=0, nsc=4)
                emit_attention(2, head_cb=op_cb(0, 4))
                emit_attention(3, head_cb=op_cb(1, 8))
                emit_outproj(1, sc0=12, nsc=4)
            elif sched == "mid2":
                emit_attention(1)
                emit_outproj(0, sc0=0, nsc=4)
                emit_attention(2)
                emit_outproj(1, sc0=8, nsc=4)
                emit_attention(3)
                emit_outproj(0, sc0=4, nsc=4)
                emit_outproj(1, sc0=12, nsc=4)
            elif sched == "mid3":
                emit_attention(1)
                emit_outproj(0, sc0=0, nsc=2)
                emit_attention(2)
                emit_outproj(0, sc0=2, nsc=2)
                emit_outproj(0, sc0=4, nsc=2)
                emit_attention(3)
                emit_outproj(0, sc0=6, nsc=2)
                emit_outproj(1, sc0=8, nsc=4)
                emit_outproj(1, sc0=12, nsc=4)
            else:
                emit_attention(1)
                emit_outproj(0)   # sc 0..7 only needs attnT of sb 0-1
                emit_attention(2)
                if KNOBS.get("op1_split", True):
                    emit_attention(3)
                    emit_outproj(1, sc0=8, nsc=4)   # sb2's rows: no attn3 dep
                    emit_outproj(1, sc0=12, nsc=4)
                else:
                    emit_attention(3)
                    emit_outproj(1)
            ph3.close()

    if not skip_compile:
        nc.compile()
    return nc


def _host_prep(inputs):
    """Build per-core input maps (shard + transpose + fold norm_w + rope-perm)."""
    hidden = np.asarray(inputs["hidden"], dtype=np.float32)
    norm_w = np.asarray(inputs["norm_w"], dtype=np.float32)
    wq = np.asarray(inputs["wq"], dtype=np.float32)
    wk = np.asarray(inputs["wk"], dtype=np.float32)
    wv = np.asarray(inputs["wv"], dtype=np.float32)
    wo = np.asarray(inputs["wo"], dtype=np.float32)

    import ml_dtypes
    BF = ml_dtypes.bfloat16

    perm = np.concatenate([np.arange(0, HD, 2), np.arange(1, HD, 2)])
    # RoPE tables exactly as the reference builds them
    freqs = 1.0 / THETA ** (np.arange(0, HD, 2)[: HD // 2].astype(np.float32) / HD)
    ang = np.outer(np.arange(S), freqs).astype(np.float32)   # [S, 64]
    cosT = np.ascontiguousarray(
        np.concatenate([np.cos(ang).T, np.cos(ang).T], axis=0).astype(np.float16)
    )
    sinT = np.ascontiguousarray(
        np.concatenate([np.sin(ang).T, np.sin(ang).T], axis=0).astype(np.float16)
    )
    Pr = np.zeros((HD, HD), np.float32)
    Pr[np.arange(64), np.arange(64) + 64] = -1.0
    Pr[np.arange(64) + 64, np.arange(64)] = 1.0
    protT = np.ascontiguousarray(Pr.T.astype(np.float16))

    hT = np.ascontiguousarray(hidden.T.astype(np.float16))
    ident = np.eye(128, dtype=np.float32).astype(BF)
    # diagonal causal masks: maskT[p, r*512 + c] = 1 if 128*r + p <= c else 0
    p_i = np.arange(128)[:, None]
    c_i = np.arange(SB)[None, :]
    maskT = np.concatenate(
        [(128 * r + p_i <= c_i).astype(np.float32) for r in range(4)], axis=1
    )
    maskT = np.ascontiguousarray(maskT.astype(BF))
    ones_col = np.ones((128, 1), np.float32)
    ones_col_bf = np.ones((128, 1), np.float32).astype(BF)
    ones_row = np.ones((1, 128), np.float32)

    in_maps = []
    for c in range(NCORES):
        wq_c = wq[QI * c: QI * (c + 1)].reshape(QH, HD, D)[:, perm, :].reshape(QI, D)
        wqT = np.ascontiguousarray((wq_c * norm_w[None, :]).T.astype(np.float16))
        wk_c = wk[HD * c: HD * (c + 1)][perm, :]
        wkT = np.ascontiguousarray((wk_c * norm_w[None, :]).T.astype(np.float16))
        wv_c = wv[HD * c: HD * (c + 1)]
        wvT = np.ascontiguousarray((wv_c * norm_w[None, :]).T.astype(np.float16))
        woT = np.ascontiguousarray(wo[:, QI * c: QI * (c + 1)].T.astype(np.float16))
        in_maps.append({
            "hT": hT, "wqT": wqT, "wkT": wkT, "wvT": wvT, "woT": woT,
            "cosT": cosT, "sinT": sinT, "protT": protT, "ident": ident,
            "ones_col": ones_col, "ones_col_bf": ones_col_bf,
            "ones_row": ones_row, "maskT": maskT,
        })
    return in_maps


def kernel(**inputs) -> np.ndarray:
    global LAST_EXEC_NS, LAST_RESULT
    if "nc" not in _CACHE:
        _CACHE["nc"] = _build()
    nc = _CACHE["nc"]
    in_maps = _host_prep(inputs)
    res = run_bass_kernel_spmd(nc, in_maps, core_ids=list(range(NCORES)))
    LAST_RESULT = res
    LAST_EXEC_NS = res.exec_time_ns
    out = res.results[0]["outp"].astype(np.float32).copy()
    for c in range(1, NCORES):
        out += res.results[c]["outp"].astype(np.float32)
    return out


# revision 47
# speedup vs baseline: 1.0020x; 1.0020x over previous
"""Trainium2 Bass kernel for nn_AttentionModule (S=2048, D=4096, H=32, KV=8, HD=128).

Sharding: tensor-parallel over heads across 8 NeuronCores. Core c owns q-heads
4c..4c+3 and kv-head c (GQA groups stay intact). Each core computes RMSNorm
(norm_w folded into weights on host, rstd computed on device), its QKV
projection shard, RoPE, causal attention for its 4 heads, and a partial output
projection against its 512 columns of wo. The host sums the 8 partial outputs
(the "all-reduce" of the tensor-parallel layout).

Dtype strategy (PE matmuls are 1 cycle/row for all 16-bit dtypes; DVE gets 2x
on 2-byte dtypes; DMA traffic halves):
 - fp16 for range-tame/precision-sensitive tensors: hidden, wq/wk/wv/wo,
   cos/sin, qT/kT (score inputs), attnT (normalized), output staging/partials.
 - bf16 for the softmax path (e, eacc, v_nat, att evac, rc broadcast): scores
   reach 18.4 so e spans e^-24..e^18 — fp16 would overflow/denormalize, bf16's
   f32-range handles it with 0.4% element error (averaged out in the matmul).
 - fp32 PSUM accumulation everywhere; the h^2 running sum (rstd) stays fp32 on
   DVE because its error enters coherently per token.

Schedule notes:
 - Phase 1 runs per s-block: QKV accumulation matmuls, then rstd + RoPE.
   PSUM: acc_ps (6 banks, left stack) closes after the last block's
   evacuations and hands its banks to sc/att/sum; misc_ps (2 banks, RIGHT
   stack) survives through the last rope, then hands banks 6-7 to o_ps.
 - emit_attention is software-pipelined per head: chunks A(h), then
   norm C(h-2), then sum B(h-1), then the att-bank evacuation (on ACT, so
   DVE bursts can't hold the att PSUM bank hostage). Within A(h) the
   score-pair loop is itself pipelined: attV of pair p is emitted after the
   scores of pair p+1 so PE isn't queued behind the exp.
 - Fine-grained causal: diagonal t-chunks only compute/mask/accumulate the
   live s-columns (>= 128r), trimming both score and attV matmul rows.
 - Emission order: attn0, attn1, op(sc0-7), attn2, attn3, op(sc8-11),
   op(sc12-15) — each outproj segment only needs already-normalized attnT.
   The final output tile group narrows [4,2,1,1] to shorten the drain tail.
 - Softmax runs in scores-transposed [t, s] layout: denominators accumulated
   on DVE (bf16 adds) + one ones-column matmul per head-block; reciprocal on
   DVE, broadcast back over partitions via a K=1 ones-row matmul.
 - Causal masking: diagonal chunks use affine_select on GPSIMD for s-blocks
   0-1 and a bf16 mask multiply on DVE for later ones.
"""
import sys

sys.path.insert(0, "/opt/trn_rl_repo")

import math
from contextlib import ExitStack

import numpy as np

import bass_rust as _bass_rust
import concourse.bacc as bacc
import concourse.mybir as mybir
import concourse.tile as tile
from concourse.bass_utils import run_bass_kernel_spmd
from concourse.hw_specs import get_activation_tables

F32R = mybir.dt.float32r
F32 = mybir.dt.float32
F16 = mybir.dt.float16
BF16 = mybir.dt.bfloat16
ALU = mybir.AluOpType
ACTF = mybir.ActivationFunctionType

S, D, H, KV, HD = 2048, 4096, 32, 8, 128
NCORES = 8
QH = H // NCORES          # 4 q heads per core
QI = QH * HD              # 512 local q dims
DC = D // 128             # 32 contraction chunks
SB = 512                  # s-block width
NSB = S // SB             # 4 s-blocks
NTC = S // 128            # 16 t-chunks
EPS = 1e-6
THETA = 50000.0
SM_SCALE = 1.0 / math.sqrt(HD)

LAST_EXEC_NS = None
LAST_RESULT = None
_CACHE = {}

# pipeline-depth knobs (tuned via timeline sim)
KNOBS = dict(hb_bufs=8, sq_act=True, t12_bufs=2, expp_bufs=6, qtmp_bufs=7,
             sc_bufs=2, wkv_bufs=3, sqp_bufs=3, hb_dc=2, interleave=True,
             mask_dve=True, csb=1, kv_dc=4, wq_dc=2, wo_cache=True,
             wop_bufs=8, obig_w=4, outb_bufs=4, early_evac=True,
             mask_pool_sb=1, ham_warmup=4, sums_dve=True, t2_pool=False,
             tail_split=True, defer_rope=False, attevac_act=True,
             attn1_first=False, op1_split=True, evac_mix=False, rb_act=True, evac_first=False,
             eacc_pool_sb=-1, tail_psum_dma=False, rope_interleave=False,
             pair_pipe=True, tail_act_dma=False, rstd_lnexp=False,
             evac_mix_last=True, wq_first=True, op_sched='mid')


class _Bacc(bacc.Bacc):
    """Bacc with activation tables reordered so the one set containing
    Exp+Ln+Copy+Square is preferred — avoids per-call ACT table reloads."""

    def insert_act_table_loads(self):
        has_activation = any(
            isinstance(i, mybir.InstActivation)
            for b in self.main_func.blocks
            for i in b.instructions
        )
        if not has_activation:
            return
        tables = list(get_activation_tables(self.m.arch).items())
        tables.sort(key=lambda kv: 0 if kv[0] == "natural_log_exp_and_others" else 1)
        _bass_rust.insert_act_table_loads(self, tables)


def _build(skip_compile=False):
    use_bacc = _Bacc if KNOBS.get("rstd_lnexp", False) else bacc.Bacc
    nc = use_bacc("TRN2", target_bir_lowering=False, debug=False)

    hT_d = nc.dram_tensor("hT", [D, S], F16, kind="ExternalInput")
    wqT_d = nc.dram_tensor("wqT", [D, QI], F16, kind="ExternalInput")
    wkT_d = nc.dram_tensor("wkT", [D, HD], F16, kind="ExternalInput")
    wvT_d = nc.dram_tensor("wvT", [D, HD], F16, kind="ExternalInput")
    woT_d = nc.dram_tensor("woT", [QI, D], F16, kind="ExternalInput")
    cos_d = nc.dram_tensor("cosT", [128, S], F16, kind="ExternalInput")
    sin_d = nc.dram_tensor("sinT", [128, S], F16, kind="ExternalInput")
    prot_d = nc.dram_tensor("protT", [128, 128], F16, kind="ExternalInput")
    ident_d = nc.dram_tensor("ident", [128, 128], BF16, kind="ExternalInput")
    onec_d = nc.dram_tensor("ones_col", [128, 1], F32R, kind="ExternalInput")
    onecb_d = nc.dram_tensor("ones_col_bf", [128, 1], BF16, kind="ExternalInput")
    oner_d = nc.dram_tensor("ones_row", [1, 128], F32R, kind="ExternalInput")
    mask_d = nc.dram_tensor("maskT", [128, 4 * SB], BF16, kind="ExternalInput")
    out_d = nc.dram_tensor("outp", [S, D], F16, kind="ExternalOutput")

    hT3 = hT_d.rearrange("(o p) s -> p o s", p=128)      # [128, 32, 2048]
    wqT3 = wqT_d.rearrange("(o p) i -> p o i", p=128)    # [128, 32, 512]
    wkT3 = wkT_d.rearrange("(o p) e -> p o e", p=128)    # [128, 32, 128]
    wvT3 = wvT_d.rearrange("(o p) e -> p o e", p=128)
    woT3 = woT_d.rearrange("(g p) j -> p g j", p=128)    # [128, 4, 4096]
    out4 = out_d.rearrange("(g p) j -> p g j", p=128)    # [128, 16, 4096]

    HB_DC = KNOBS.get("hb_dc", 2)  # hT chunks per DMA

    with tile.TileContext(nc) as tc:
        with ExitStack() as root:
            consts = root.enter_context(tc.tile_pool(name="consts", bufs=1))
            persist = root.enter_context(tc.tile_pool(name="persist", bufs=1))

            onec_t = consts.tile([128, 1], F32R, tag="onec")
            onecb_t = consts.tile([128, 1], BF16, tag="onecb")
            oner_t = consts.tile([1, 128], F32R, tag="oner")
            eps_t = consts.tile([1, 1], F32, tag="eps")
            nc.vector.memset(eps_t, EPS)

            qT_all = persist.tile([128, QH, S], F16, tag="qT")
            kT_all = persist.tile([128, S], F16, tag="kT")
            v_nat = persist.tile([128, NTC, HD], BF16, tag="vn")

            # rope constants + scratch outlive phase 1 (sb3's rope is
            # deferred into the attention region)
            c1 = root.enter_context(tc.tile_pool(name="c1", bufs=1))
            cos_t = c1.tile([128, S], F16, tag="cos")
            sin_t = c1.tile([128, S], F16, tag="sin")
            prot_t = c1.tile([128, 128], F16, tag="prot")
            ident_t = c1.tile([128, 128], BF16, tag="ident")
            c1_loaded = [False]
            scr2 = root.enter_context(tc.tile_pool(name="scr2", bufs=2))

            # misc_ps on the RIGHT psum stack: outlives acc_ps (left) through
            # the deferred rope, then hands banks 6-7 to o_ps
            misc_stack = ExitStack()
            misc_ps = misc_stack.enter_context(
                tc.tile_pool(name="misc_ps", bufs=2, space="PSUM", side="right")
            )

            # --- rstd/rope helpers (used both in-loop and deferred) ---
            def emit_rstd(sb, sqr, sq_last=None):
                ssl = slice(SB * sb, SB * (sb + 1))
                ms_ps = misc_ps.tile([1, SB], F32, tag="misc", name=f"ms_ps{sb}")
                if sq_last is None:
                    nc.tensor.matmul(ms_ps, onec_t, sqr, start=True, stop=True)
                else:
                    # split: bulk term ready one DVE-add earlier than the tail
                    nc.tensor.matmul(ms_ps, onec_t, sqr, start=True, stop=False)
                    nc.tensor.matmul(ms_ps, onec_t, sq_last, start=False,
                                     stop=True)
                lnt = scr2.tile([1, SB], F32, tag="lnt", bufs=1)
                rstd = scr2.tile([1, SB], F32R, tag="rstd", bufs=1)
                if KNOBS.get("rstd_lnexp", True):
                    # rstd = exp(-0.5 ln(ms/D + eps)): both funcs live in the
                    # natural_log_exp set, and the whole chain stays on ACT so
                    # the rb broadcast never waits on the DVE queue
                    nc.scalar.activation(
                        out=lnt, in_=ms_ps, func=ACTF.Ln, scale=1.0 / D,
                        bias=eps_t
                    )
                    with nc.allow_low_precision(reason="rstd row fp32r"):
                        nc.scalar.activation(
                            out=rstd, in_=lnt, func=ACTF.Exp, scale=-0.5
                        )
                else:
                    nc.scalar.activation(
                        out=lnt, in_=ms_ps, func=ACTF.Sqrt, scale=1.0 / D,
                        bias=eps_t
                    )
                    with nc.allow_low_precision(reason="rstd row fp32r"):
                        nc.vector.reciprocal(out=rstd, in_=lnt.bitcast(F32R))
                rb_ps = misc_ps.tile([128, SB], F32, tag="misc", name=f"rb_ps{sb}")
                nc.tensor.matmul(rb_ps, oner_t, rstd, start=True, stop=True)
                rb_sb = scr2.tile([128, SB], F16, tag="rb_sb",
                                  bufs=KNOBS.get("csb", 2))
                (nc.scalar.copy if KNOBS.get("rb_act", True)
                 else nc.vector.tensor_copy)(out=rb_sb, in_=rb_ps)
                cosrb = scr2.tile([128, SB], F16, tag="cosrb",
                                  bufs=KNOBS.get("csb", 2))
                nc.vector.tensor_tensor(cosrb, cos_t[:, ssl], rb_sb, ALU.mult)
                sinrb = scr2.tile([128, SB], F16, tag="sinrb",
                                  bufs=KNOBS.get("csb", 2))
                nc.vector.tensor_tensor(sinrb, sin_t[:, ssl], rb_sb, ALU.mult)
                return rb_sb, cosrb, sinrb

            def emit_rope_q(sb, i, qtmp, cosrb, sinrb):
                ssl = slice(SB * sb, SB * (sb + 1))
                rot_ps = misc_ps.tile([128, SB], F32, tag="misc",
                                      name=f"rot_q{i}_{sb}")
                nc.tensor.matmul(rot_ps, prot_t, qtmp, start=True, stop=True)
                t1 = scr2.tile([128, SB], F16, tag="t1", bufs=KNOBS["t12_bufs"])
                nc.vector.tensor_tensor(t1, qtmp, cosrb, ALU.mult)
                t2 = scr2.tile([128, SB], F16, tag="t2", bufs=KNOBS["t12_bufs"])
                (nc.gpsimd if KNOBS.get("t2_pool", False) else nc.vector
                 ).tensor_tensor(t2, rot_ps, sinrb, ALU.mult)
                nc.vector.tensor_tensor(qT_all[:, i, ssl], t1, t2, ALU.add)

            def emit_rope_k(sb, ktmp, cosrb, sinrb):
                ssl = slice(SB * sb, SB * (sb + 1))
                rot_ps = misc_ps.tile([128, SB], F32, tag="misc",
                                      name=f"rot_k{sb}")
                nc.tensor.matmul(rot_ps, prot_t, ktmp, start=True, stop=True)
                t1 = scr2.tile([128, SB], F16, tag="t1",
                               bufs=KNOBS["t12_bufs"], name="t1k")
                nc.vector.tensor_tensor(t1, ktmp, cosrb, ALU.mult)
                t2 = scr2.tile([128, SB], F16, tag="t2",
                               bufs=KNOBS["t12_bufs"], name="t2k")
                (nc.gpsimd if KNOBS.get("t2_pool", False) else nc.vector
                 ).tensor_tensor(t2, rot_ps, sinrb, ALU.mult)
                nc.vector.tensor_tensor(kT_all[:, ssl], t1, t2, ALU.add)

            def emit_v(sb, vtmp, rb_sb):
                vsc = scr2.tile([128, SB], BF16, tag="vsc",
                                bufs=KNOBS.get("csb", 2))
                nc.vector.tensor_tensor(vsc, vtmp, rb_sb, ALU.mult)
                for j in range(SB // 128):
                    tcx = (SB // 128) * sb + j
                    vtr_ps = misc_ps.tile([128, 128], BF16, tag="misc",
                                          name=f"vtr{tcx}")
                    nc.tensor.transpose(
                        vtr_ps, vsc[:, 128 * j: 128 * (j + 1)], ident_t
                    )
                    nc.vector.tensor_copy(out=v_nat[:, tcx, :], in_=vtr_ps)

            # ------------- Phase 1: QKV projections + rstd + RoPE -------------
            saved = {}
            with ExitStack() as ph1:
                wqp = ph1.enter_context(tc.tile_pool(name="wqp", bufs=1))
                wq_t = wqp.tile([128, DC, QI], F16, tag="wqr")
                wkvp = ph1.enter_context(tc.tile_pool(name="wkvp", bufs=KNOBS["wkv_bufs"]))
                hb = ph1.enter_context(tc.tile_pool(name="hb", bufs=KNOBS["hb_bufs"]))
                sqp = ph1.enter_context(tc.tile_pool(name="sqp", bufs=KNOBS["sqp_bufs"]))
                scrA = ph1.enter_context(tc.tile_pool(name="scrA", bufs=2))
                if KNOBS.get("acc_split", True):
                    # k/v banks (released last -> att/sum) below q banks
                    # (released first -> sc): scores start after only the
                    # four q evacuations instead of all six
                    acca_st = ExitStack()
                    acc_kv = acca_st.enter_context(
                        tc.tile_pool(name="acc_kv", bufs=1, space="PSUM"))
                    accb_st = ExitStack()
                    acc_q = accb_st.enter_context(
                        tc.tile_pool(name="acc_q", bufs=1, space="PSUM"))
                else:
                    acc_ps = ph1.enter_context(
                        tc.tile_pool(name="acc_ps", bufs=1, space="PSUM")
                    )
                    acc_kv = acc_q = acc_ps
                    acca_st = accb_st = None

                if KNOBS.get("ham_warmup", 0):
                    # PE p-state ramp warm-up: dummy matmuls on a zeroed tile
                    # during the initial DMA wait so real matmuls start at
                    # full clock (the cost model tracks a 3us ramp of
                    # continuous PE activity).
                    wu = scr2.tile([128, SB], F16, tag="qtmp",
                                   bufs=KNOBS["qtmp_bufs"], name="warmup_src")
                    nc.vector.memset(wu, 0.0)
                    wu_ps = misc_ps.tile([128, SB], F32, tag="misc", name="wu_ps")
                    for _w in range(KNOBS["ham_warmup"]):
                        nc.tensor.matmul(wu_ps, wu[:, :128], wu,
                                         start=(_w == 0),
                                         stop=(_w == KNOBS["ham_warmup"] - 1))

                pending_rope = []
                for sb in range(NSB):
                    ssl = slice(SB * sb, SB * (sb + 1))
                    q_ps = [
                        acc_q.tile([128, SB], F32, tag=f"q{i}", name=f"q_ps{i}")
                        for i in range(QH)
                    ]
                    k_ps = acc_kv.tile([128, SB], F32, tag="k")
                    v_ps = acc_kv.tile([128, SB], F32, tag="v")
                    sqacc = scrA.tile([128, SB], F32, tag="sqacc", bufs=2)
                    sqr = scrA.tile([128, SB], F32R, tag="sqr", bufs=1)
                    sq_last = [None]
                    KV_DC = KNOBS.get("kv_dc", 4)  # wk/wv chunk width
                    WQ_DC = KNOBS.get("wq_dc", 2)  # wq load width (sb 0)
                    for hc in range(DC // HB_DC):
                        if sb == 0 and hc == 0 and KNOBS.get("wq_first", True):
                            # first q-matmul needs wq chunk 0: issue it ahead
                            # of the h/wk/wv stream
                            nc.sync.dma_start(out=wq_t[:, 0:WQ_DC, :],
                                              in_=wqT3[:, 0:WQ_DC, :])
                        ht2 = hb.tile([128, HB_DC, SB], F16, tag="h")
                        nc.sync.dma_start(out=ht2, in_=hT3[:, HB_DC*hc:HB_DC*(hc+1), ssl])
                        if (HB_DC * hc) % KV_DC == 0:
                            kc0 = HB_DC * hc
                            wkc = wkvp.tile([128, KV_DC, HD], F16, tag="wk2")
                            nc.sync.dma_start(
                                out=wkc, in_=wkT3[:, kc0:kc0+KV_DC, :])
                            wvc = wkvp.tile([128, KV_DC, HD], F16, tag="wv2")
                            nc.sync.dma_start(
                                out=wvc, in_=wvT3[:, kc0:kc0+KV_DC, :])
                        if (KNOBS.get("rope_interleave", True)
                                and pending_rope and hc % 2 == 0):
                            pending_rope.pop(0)()
                        for j in range(HB_DC):
                            dc = HB_DC * hc + j
                            ht = ht2[:, j, :]
                            if sb == 0 and dc % WQ_DC == 0 and not (
                                    dc == 0 and KNOBS.get("wq_first", True)):
                                nc.sync.dma_start(out=wq_t[:, dc:dc+WQ_DC, :],
                                                  in_=wqT3[:, dc:dc+WQ_DC, :])
                            wqc = wq_t[:, dc, :]
                            if sb == 0 and dc == 8 and not c1_loaded[0]:
                                nc.sync.dma_start(out=cos_t, in_=cos_d[:, :])
                                nc.sync.dma_start(out=sin_t, in_=sin_d[:, :])
                                nc.sync.dma_start(out=prot_t, in_=prot_d[:, :])
                                nc.sync.dma_start(out=ident_t, in_=ident_d[:, :])
                                nc.sync.dma_start(out=onec_t, in_=onec_d[:, :])
                                nc.sync.dma_start(out=onecb_t, in_=onecb_d[:, :])
                                nc.sync.dma_start(out=oner_t, in_=oner_d[:, :])
                                c1_loaded[0] = True
                            sq = sqp.tile([128, SB],
                                          F32R if dc == DC - 1 else F32,
                                          tag="sq")
                            if KNOBS["sq_act"]:
                                nc.scalar.activation(out=sq, in_=ht, func=ACTF.Square)
                            else:
                                nc.vector.tensor_tensor(sq, ht, ht, ALU.mult)
                            if dc == 0:
                                nc.vector.tensor_copy(out=sqacc, in_=sq)
                            elif dc == DC - 1:
                                sq_last[0] = sq
                            elif dc == DC - 2:
                                nc.vector.tensor_tensor(sqr, sqacc, sq, ALU.add)
                            else:
                                nc.vector.tensor_tensor(sqacc, sqacc, sq, ALU.add)
                            for i in range(QH):
                                nc.tensor.matmul(
                                    q_ps[i],
                                    wqc[:, 128 * i: 128 * (i + 1)],
                                    ht,
                                    start=(dc == 0),
                                    stop=(dc == DC - 1),
                                )
                            nc.tensor.matmul(
                                k_ps, wkc[:, dc % KV_DC, :], ht,
                                start=(dc == 0), stop=(dc == DC - 1),
                            )
                            nc.tensor.matmul(
                                v_ps, wvc[:, dc % KV_DC, :], ht,
                                start=(dc == 0), stop=(dc == DC - 1),
                            )
                    # rstd + PSUM evacuation. Evacs are plain copies with no
                    # rstd dependency, so the next s-block's accumulation
                    # starts immediately; rstd is folded into per-block
                    # cos/sin tables instead.
                    if KNOBS.get("evac_first", True):
                        rb_sb = cosrb = sinrb = None  # rstd emitted after evacs
                    else:
                        rb_sb, cosrb, sinrb = emit_rstd(sb, sqr, sq_last[0])
                    mix_here = (KNOBS.get("evac_mix", True)
                                or (sb == NSB - 1
                                    and KNOBS.get("evac_mix_last", True)))
                    if mix_here:
                        # GPSIMD cannot read PSUM: alternate DVE/ACT only
                        evac_engines = [nc.vector.tensor_copy, nc.scalar.copy,
                                        nc.vector.tensor_copy, nc.scalar.copy,
                                        nc.vector.tensor_copy, nc.scalar.copy]
                    else:
                        evac_engines = [nc.vector.tensor_copy] * 6
                    qtmps = []
                    for i in range(QH):
                        qtmp = scr2.tile([128, SB], F16, tag="qtmp",
                                         bufs=KNOBS["qtmp_bufs"])
                        evac_engines[i](out=qtmp, in_=q_ps[i])
                        qtmps.append(qtmp)
                    ktmp = scr2.tile([128, SB], F16, tag="qtmp",
                                     bufs=KNOBS["qtmp_bufs"], name="ktmp")
                    evac_engines[4](out=ktmp, in_=k_ps)
                    vtmp = scr2.tile([128, SB], BF16, tag="qtmp",
                                     bufs=KNOBS["qtmp_bufs"], name="vtmp")
                    evac_engines[5](out=vtmp, in_=v_ps)
                    if KNOBS.get("evac_first", True):
                        rb_sb, cosrb, sinrb = emit_rstd(sb, sqr, sq_last[0])
                    if sb == NSB - 1 and KNOBS.get("defer_rope", True):
                        saved = dict(qtmps=qtmps, ktmp=ktmp, vtmp=vtmp,
                                     rb_sb=rb_sb, cosrb=cosrb, sinrb=sinrb)
                    elif sb < NSB - 1 and KNOBS.get("rope_interleave", True):
                        # spread this block's rope into the next block's
                        # accumulation loop (DVE has slack there)
                        import functools
                        for i in range(QH):
                            pending_rope.append(functools.partial(
                                emit_rope_q, sb, i, qtmps[i], cosrb, sinrb))
                        pending_rope.append(functools.partial(
                            emit_rope_k, sb, ktmp, cosrb, sinrb))
                        pending_rope.append(functools.partial(
                            emit_v, sb, vtmp, rb_sb))
                    else:
                        for i in range(QH):
                            emit_rope_q(sb, i, qtmps[i], cosrb, sinrb)
                        emit_rope_k(sb, ktmp, cosrb, sinrb)
                        emit_v(sb, vtmp, rb_sb)
                for fn in pending_rope:
                    fn()
                pending_rope = []

            # ph1 closed: release q banks (-> sc) then k/v banks (-> att/sum)
            if KNOBS.get("acc_split", True):
                accb_st.close()
                acca_st.close()
            persist2 = root.enter_context(tc.tile_pool(name="persist2", bufs=1))
            attnT = persist2.tile([128, QH, S], F16, tag="attnT")
            mask_t = persist2.tile([128, 4, SB], BF16, tag="mask")
            nc.sync.dma_start(out=mask_t, in_=mask_d.rearrange("p (r s) -> p r s", s=SB))

            outb = root.enter_context(tc.tile_pool(name="outb", bufs=KNOBS.get("outb_bufs", 2)))
            wop = root.enter_context(tc.tile_pool(name="wop", bufs=KNOBS.get("wop_bufs", 2)))

            # ------------- Phase 3+4 interleaved ------------------------------
            ph3 = ExitStack()
            sc_ps_p = ph3.enter_context(
                tc.tile_pool(name="sc_ps", bufs=KNOBS["sc_bufs"], space="PSUM")
            )
            att_ps_p = ph3.enter_context(
                tc.tile_pool(name="att_ps", bufs=KNOBS.get("att_bufs", 1), space="PSUM")
            )
            sum_ps_p = ph3.enter_context(
                tc.tile_pool(name="sum_ps", bufs=1, space="PSUM")
            )
            expp = ph3.enter_context(tc.tile_pool(name="expp", bufs=KNOBS["expp_bufs"]))
            scr3 = ph3.enter_context(tc.tile_pool(name="scr3", bufs=2))

            o_holder = [None]

            def emit_attention(sb, head_cb=None):
                ssl = slice(SB * sb, SB * (sb + 1))
                n_tc = (SB // 128) * (sb + 1)
                SCP = 2 if KNOBS.get("sc_pair", True) else 1
                st = {}  # per-head pipeline state

                def stage_chunks(h):
                    att_ps = att_ps_p.tile([128, SB], F32, tag="att",
                                           name=f"att{h}_{sb}")
                    eacc = scr3.tile([128, SB], BF16, tag="eacc", bufs=2,
                                     name=f"eacc{h}_{sb}")
                    def consume_pair(e_pair, rr):
                        # mask + attV + denominator accumulation for a pair
                        for u in range(len(rr)):
                            tcx, r, csl = rr[u]
                            e_sb = e_pair[:, u, csl]
                            if r >= 0:
                                # diagonal chunk: zero where t > s; dense early
                                # blocks go to idle GPSIMD, late ones to DVE
                                if sb <= KNOBS.get("mask_pool_sb", -1):
                                    nc.gpsimd.affine_select(
                                        e_sb, e_sb,
                                        pattern=[[1, csl.stop - csl.start]],
                                        compare_op=ALU.is_ge,
                                        fill=0.0,
                                        base=-(128 * r) + csl.start,
                                        channel_multiplier=-1,
                                    )
                                else:
                                    nc.vector.tensor_tensor(
                                        e_sb, e_sb, mask_t[:, r, csl], ALU.mult
                                    )
                            nc.tensor.matmul(
                                att_ps[:, csl], v_nat[:, tcx, :], e_sb,
                                start=(tcx == 0), stop=(tcx == n_tc - 1),
                            )
                            if tcx == 0:
                                nc.vector.tensor_copy(out=eacc, in_=e_sb)
                            else:
                                nc.vector.tensor_tensor(
                                    eacc[:, csl], eacc[:, csl], e_sb, ALU.add)

                    pending = []  # (e_pair, rr) whose attV is not yet emitted
                    for tp in range(n_tc // SCP):
                        # paired scores tiles -> one wide exp; diagonal chunks
                        # only compute the live s-columns (>= 128r). attV of
                        # pair p is emitted after scores of pair p+1 so PE
                        # isn't queued behind the exp.
                        sc_ps = sc_ps_p.tile([128, SCP, SB], F32, tag="sc",
                                             name=f"sc{h}_{sb}_{tp}")
                        e_pair = expp.tile([128, SCP, SB], BF16, tag="e",
                                           name=f"e{h}_{sb}_{tp}")
                        rr = []
                        for u in range(SCP):
                            tcx = SCP * tp + u
                            r = tcx - (SB // 128) * sb
                            csl = slice(128 * r, SB) if r > 0 else slice(0, SB)
                            rr.append((tcx, r, csl))
                            nc.tensor.matmul(
                                sc_ps[:, u, csl],
                                kT_all[:, 128 * tcx: 128 * (tcx + 1)],
                                qT_all[:, h, SB * sb + csl.start:
                                       SB * sb + csl.stop],
                                start=True, stop=True,
                            )
                        if rr[0][1] > 0:
                            # all-diagonal pair: exp only the live columns
                            for u in range(SCP):
                                tcx, r, csl = rr[u]
                                nc.scalar.activation(
                                    out=e_pair[:, u, csl], in_=sc_ps[:, u, csl],
                                    func=ACTF.Exp, scale=SM_SCALE
                                )
                        else:
                            nc.scalar.activation(
                                out=e_pair, in_=sc_ps, func=ACTF.Exp,
                                scale=SM_SCALE
                            )
                        if KNOBS.get("pair_pipe", True):
                            pending.append((e_pair, rr))
                            if len(pending) > KNOBS.get("pair_depth", 1):
                                consume_pair(*pending.pop(0))
                        else:
                            consume_pair(e_pair, rr)
                    for pp in pending:
                        consume_pair(*pp)
                    pending = []
                    st[h] = dict(att_ps=att_ps, eaccr=eacc)

                def stage_evac(h):
                    # unnormalized numerator off PSUM (bf16 for range); on ACT
                    # so a busy DVE can't hold the att bank hostage
                    att_sb = scr3.tile([128, SB], BF16, tag="attsb", bufs=2,
                                       name=f"attsb{h}_{sb}")
                    if KNOBS.get("attevac_act", True):
                        nc.scalar.copy(out=att_sb, in_=st[h]["att_ps"])
                    else:
                        nc.vector.tensor_copy(out=att_sb, in_=st[h]["att_ps"])
                    st[h]["att_sb"] = att_sb

                def stage_sum(h):
                    sum_ps = sum_ps_p.tile([1, SB], F32, tag="sumrc",
                                           name=f"sum{h}_{sb}")
                    nc.tensor.matmul(sum_ps, onecb_t, st[h]["eaccr"],
                                     start=True, stop=True)
                    st[h]["sum_ps"] = sum_ps

                def stage_norm(h):
                    rcv = scr3.tile([1, SB], F32R, tag="rcv", bufs=2,
                                    name=f"rcv{h}_{sb}")
                    with nc.allow_low_precision(reason="softmax recip row"):
                        nc.vector.reciprocal(out=rcv,
                                             in_=st[h]["sum_ps"].bitcast(F32R))
                    rc_ps = sum_ps_p.tile([128, SB], F32, tag="sumrc",
                                          name=f"rc{h}_{sb}")
                    nc.tensor.matmul(rc_ps, oner_t, rcv, start=True, stop=True)
                    rc_sb = scr3.tile([128, SB], BF16, tag="rcsb", bufs=2,
                                      name=f"rcsb{h}_{sb}")
                    nc.vector.tensor_copy(out=rc_sb, in_=rc_ps)
                    nc.vector.tensor_tensor(
                        attnT[:, h, ssl], st[h]["att_sb"], rc_sb, ALU.mult
                    )

                for h in range(QH):
                    stage_chunks(h)
                    if h >= 2:
                        stage_norm(h - 2)
                    if h >= 1:
                        stage_sum(h - 1)
                    stage_evac(h)
                    if head_cb is not None:
                        head_cb(h)
                stage_norm(QH - 2)
                stage_sum(QH - 1)
                stage_norm(QH - 1)

            woc_cache = {}

            def emit_outproj(g, sc0=None, nsc=8, jts=None):
                if sc0 is None:
                    sc0 = 8 * g
                OBW = KNOBS.get("obig_w", 4)  # sc-tiles per out staging/DMA
                for jt in (range(D // SB) if jts is None else jts):
                    jsl = slice(SB * jt, SB * (jt + 1))
                    if KNOBS.get("wo_cache", False):
                        if jt not in woc_cache:
                            woc = wop.tile([128, QH, SB], F16, tag="wo",
                                           name=f"wo{jt}")
                            nc.sync.dma_start(out=woc, in_=woT3[:, :, jsl])
                            woc_cache[jt] = woc
                        woc = woc_cache[jt]
                    else:
                        woc = wop.tile([128, QH, SB], F16, tag="wo",
                                       name=f"wo{jt}_{g}")
                        nc.sync.dma_start(out=woc, in_=woT3[:, :, jsl])
                    if sc0 + nsc == 16 and jt == D // SB - 1 and KNOBS.get("tail_split", True):
                        groups = [OBW] * ((nsc - 4) // OBW) + [2, 1, 1]
                    else:
                        assert nsc % OBW == 0 and nsc // OBW > 0, (nsc, OBW)
                        groups = [OBW] * (nsc // OBW)
                    sc_base = sc0
                    for q, gw in enumerate(groups):
                        o_big = outb.tile([128, OBW, SB], F16, tag="obig",
                                          name=f"ob{jt}_{g}_{q}")
                        last_single = (gw == 1 and q == len(groups) - 1
                                       and KNOBS.get("tail_psum_dma", False))
                        for si in range(gw):
                            sc = sc_base + si
                            o_ps = o_holder[0].tile([128, SB], F32, tag="o",
                                                    name=f"o{jt}_{sc}")
                            for h in range(QH):
                                nc.tensor.matmul(
                                    o_ps,
                                    attnT[:, h, 128 * sc: 128 * (sc + 1)],
                                    woc[:, h, :],
                                    start=(h == 0), stop=(h == QH - 1),
                                )
                            if last_single:
                                continue
                            if si % 2 == 0:
                                nc.vector.tensor_copy(out=o_big[:, si, :], in_=o_ps)
                            else:
                                nc.scalar.copy(out=o_big[:, si, :], in_=o_ps)
                        if last_single:
                            nc.sync.dma_start(
                                out=out4[:, sc_base: sc_base + 1, jsl],
                                in_=o_ps,
                            )
                        else:
                            eng = (nc.scalar
                                   if (gw <= 2 and KNOBS.get("tail_act_dma", True))
                                   else nc.sync)
                            eng.dma_start(
                                out=out4[:, sc_base: sc_base + gw, jsl],
                                in_=o_big[:, :gw, :]
                            )
                        sc_base += gw

            if KNOBS.get("defer_rope", True):
                def rope3_head(h):
                    emit_rope_q(NSB - 1, h, saved["qtmps"][h],
                                saved["cosrb"], saved["sinrb"])
                emit_attention(0, head_cb=rope3_head)
                emit_rope_k(NSB - 1, saved["ktmp"], saved["cosrb"],
                            saved["sinrb"])
                emit_v(NSB - 1, saved["vtmp"], saved["rb_sb"])
                misc_stack.close()
                o_holder[0] = root.enter_context(
                    tc.tile_pool(name="o_ps", bufs=2, space="PSUM", side="right")
                )
                emit_attention(1)
            elif KNOBS.get("attn1_first", True):
                # attention(1) has 2x the matmul volume of attention(0):
                # lead with it so sb3's in-loop rope chain hides behind PE work
                emit_attention(1)
                misc_stack.close()
                o_holder[0] = root.enter_context(
                    tc.tile_pool(name="o_ps", bufs=2, space="PSUM", side="right")
                )
                emit_attention(0)
            else:
                emit_attention(0)
                misc_stack.close()
                o_holder[0] = root.enter_context(
                    tc.tile_pool(name="o_ps", bufs=2, space="PSUM", side="right")
                )
                sched = KNOBS.get("op_sched", "late")
            if sched == "early":
                # each outproj segment right after the attention block that
                # completes its attnT rows: fills transition idle with PE work
                emit_outproj(0, sc0=0, nsc=4)
                emit_attention(1)
                emit_outproj(0, sc0=4, nsc=4)
                emit_attention(2)
                emit_outproj(1, sc0=8, nsc=4)
                emit_attention(3)
                emit_outproj(1, sc0=12, nsc=4)
            elif sched == "mid":
                emit_attention(1)
                emit_outproj(0, sc0=0, nsc=4)
                emit_attention(2)
                emit_outproj(0, sc0=4, nsc=4)
                emit_attention(3)
                emit_outproj(1, sc0=8, nsc=4)
                emit_outproj(1, sc0=12, nsc=4)
            elif sched == "fine":
                def op_cb(g, sc0):
                    def cb(h):
                        emit_outproj(g, sc0=sc0, nsc=4, jts=[2 * h, 2 * h + 1])
                    return cb
                emit_attention(1, head_cb=op_cb(0, 0))
                emit_attention(2, head_cb=op_cb(0, 4))
                emit_attention(3, head_cb=op_cb(1, 8))
                emit_outproj(1, sc0=12, nsc=4)
            elif sched == "fine3":
                # op filler callbacks only where attention is ACT-bound
                def op_cb(g, sc0):
                    def cb(h):
                        emit_outproj(g, sc0=sc0, nsc=4, jts=[2 * h, 2 * h + 1])
                    return cb
                emit_attention(1)
                emit_outproj(0, sc0=0, nsc=4)
                emit_attention(2)
                emit_attention(3, head_cb=op_cb(0, 4))
                emit_outproj(1, sc0=8, nsc=4)
                emit_outproj(1, sc0=12, nsc=4)
            elif sched == "fine23":
                def op_cb(g, sc0):
                    def cb(h):
                        emit_outproj(g, sc0=sc0, nsc=4, jts=[2 * h, 2 * h + 1])
                    return cb
                emit_attention(1)
                emit_outproj(0, sc0=0, nsc=4)
                emit_attention(2, head_cb=op_cb(0, 4))
                emit_attention(3, head_cb=op_cb(1, 8))
                emit_outproj(1, sc0=12, nsc=4)
            elif sched == "mid2":
                emit_attention(1)
                emit_outproj(0, sc0=0, nsc=4)
                emit_attention(2)
                emit_outproj(1, sc0=8, nsc=4)
                emit_attention(3)
                emit_outproj(0, sc0=4, nsc=4)
                emit_outproj(1, sc0=12, nsc=4)
            elif sched == "mid3":
                emit_attention(1)
                emit_outproj(0, sc0=0, nsc=2)
                emit_attention(2)
                emit_outproj(0, sc0=2, nsc=2)
                emit_outproj(0, sc0=4, nsc=2)
                emit_attention(3)
                emit_outproj(0, sc0=6, nsc=2)
                emit_outproj(1, sc0=8, nsc=4)
                emit_outproj(1, sc0=12, nsc=4)
            else:
                emit_attention(1)
                emit_outproj(0)   # sc 0..7 only needs attnT of sb 0-1
                emit_attention(2)
                if KNOBS.get("op1_split", True):
                    emit_attention(3)
                    emit_outproj(1, sc0=8, nsc=4)   # sb2's rows: no attn3 dep
                    emit_outproj(1, sc0=12, nsc=4)
                else:
                    emit_attention(3)
                    emit_outproj(1)
            ph3.close()

    if not skip_compile:
        nc.compile()
    return nc


def _host_prep(inputs):
    """Build per-core input maps (shard + transpose + fold norm_w + rope-perm)."""
    hidden = np.asarray(inputs["hidden"], dtype=np.float32)
    norm_w = np.asarray(inputs["norm_w"], dtype=np.float32)
    wq = np.asarray(inputs["wq"], dtype=np.float32)
    wk = np.asarray(inputs["wk"], dtype=np.float32)
    wv = np.asarray(inputs["wv"], dtype=np.float32)
    wo = np.asarray(inputs["wo"], dtype=np.float32)

    import ml_dtypes
    BF = ml_dtypes.bfloat16

    perm = np.concatenate([np.arange(0, HD, 2), np.arange(1, HD, 2)])
    # RoPE tables exactly as the reference builds them
    freqs = 1.0 / THETA ** (np.arange(0, HD, 2)[: HD // 2].astype(np.float32) / HD)
    ang = np.outer(np.arange(S), freqs).astype(np.float32)   # [S, 64]
    cosT = np.ascontiguousarray(
        np.concatenate([np.cos(ang).T, np.cos(ang).T], axis=0).astype(np.float16)
    )
    sinT = np.ascontiguousarray(
        np.concatenate([np.sin(ang).T, np.sin(ang).T], axis=0).astype(np.float16)
    )
    Pr = np.zeros((HD, HD), np.float32)
    Pr[np.arange(64), np.arange(64) + 64] = -1.0
    Pr[np.arange(64) + 64, np.arange(64)] = 1.0
    protT = np.ascontiguousarray(Pr.T.astype(np.float16))

    hT = np.ascontiguousarray(hidden.T.astype(np.float16))
    ident = np.eye(128, dtype=np.float32).astype(BF)
    # diagonal causal masks: maskT[p, r*512 + c] = 1 if 128*r + p <= c else 0
    p_i = np.arange(128)[:, None]
    c_i = np.arange(SB)[None, :]
    maskT = np.concatenate(
        [(128 * r + p_i <= c_i).astype(np.float32) for r in range(4)], axis=1
    )
    maskT = np.ascontiguousarray(maskT.astype(BF))
    ones_col = np.ones((128, 1), np.float32)
    ones_col_bf = np.ones((128, 1), np.float32).astype(BF)
    ones_row = np.ones((1, 128), np.float32)

    in_maps = []
    for c in range(NCORES):
        wq_c = wq[QI * c: QI * (c + 1)].reshape(QH, HD, D)[:, perm, :].reshape(QI, D)
        wqT = np.ascontiguousarray((wq_c * norm_w[None, :]).T.astype(np.float16))
        wk_c = wk[HD * c: HD * (c + 1)][perm, :]
        wkT = np.ascontiguousarray((wk_c * norm_w[None, :]).T.astype(np.float16))
        wv_c = wv[HD * c: HD * (c + 1)]
        wvT = np.ascontiguousarray((wv_c * norm_w[None, :]).T.astype(np.float16))
        woT = np.ascontiguousarray(wo[:, QI * c: QI * (c + 1)].T.astype(np.float16))
        in_maps.append({
            "hT": hT, "wqT": wqT, "wkT": wkT, "wvT": wvT, "woT": woT,
            "cosT": cosT, "sinT": sinT, "protT": protT, "ident": ident,
            "ones_col": ones_col, "ones_col_bf": ones_col_bf,
            "ones_row": ones_row, "maskT": maskT,
        })
    return in_maps


def kernel(**inputs) -> np.ndarray:
    global LAST_EXEC_NS, LAST_RESULT
    if "nc" not in _CACHE:
        _CACHE["nc"] = _build()
    nc = _CACHE["nc"]
    in_maps = _host_prep(inputs)
    res = run_bass_kernel_spmd(nc, in_maps, core_ids=list(range(NCORES)))
    LAST_RESULT = res
    LAST_EXEC_NS = res.exec_time_ns
    out = res.results[0]["outp"].astype(np.float32).copy()
    for c in range(1, NCORES):
        out += res.results[c]["outp"].astype(np.float32)
    return out
